# revision 5
# baseline (speedup 1.0000x reference)
"""Trainium2 Bass kernel for nn_EnhancedQuantumPINN.

Reference computation (per batch element b of B=131072):
    h      = tanh(xy @ W1 + b1)                  # [16]
    theta  = tanh(h @ W2 + b2)                   # [40] rotation angles
    psi    = 4-qubit circuit:  per qubit: H, RY(pi*x), RZ(pi*y)  (init)
             then 5 layers of {RX(th), RY(th)} per qubit + CZ ring
    q      = |psi|^2 @ Z_signs.T                 # [4]
    out    = tanh(q @ W3 + b3) @ W4 + b4         # [1]

Kernel strategy (pure data parallelism, 8 cores x 16384 elements):
  * batch-major layout: element b = m*128 + lane -> partition=lane, block=m.
    State tensor [128, 32*M] f32: col = comp*M + m, comp<16 are Re(amp),
    comp>=16 are Im(amp).
  * init state in closed form (product state, amplitudes depend only on
    popcount): amp_k = r0^(4-n) r1^n e^{i*pi*y*(n-2)} (global 1/4 folded
    into the readout scale).
  * each rotation gate normalized: psi' = cos(t/2) * (psi + tan(t/2)*P psi)
    where P is a signed amp-permutation; the cos prefactors are multiplied
    into one scalar C per element and applied to the probabilities.
    tan via odd degree-7 polynomial (|theta/2| <= 0.5, rel err ~1e-6).
  * CZ ring per layer = single fixed +-1 diagonal multiply.
  * readout: ACT Square -> Walsh sums (adds/subs over amp blocks) -> head
    MLP with per-partition tensor_scalar ops.
"""

import os
import sys

import numpy as np

for _p in ("/opt/trn_rl_repo", "/root/.axon_site/_ro/trn_rl_repo"):
    if os.path.isdir(_p) and _p not in sys.path:
        sys.path.append(_p)

import concourse.bass as bass
import concourse.bacc as bacc
import concourse.mybir as mybir
from concourse import masks, tile
from concourse import bass_utils

F32 = mybir.dt.float32
BF16 = mybir.dt.bfloat16
AF = mybir.ActivationFunctionType
OP = mybir.AluOpType

N_CORES = 8
B_FULL = 131072
N = B_FULL // N_CORES          # 16384 elements per core
M = N // 128                   # 128 column blocks
CH = 8                         # m-blocks per front-end chunk (8*128 = 1024 elems)
NCHUNK = M // CH
NANG = 40                      # rotation angles per element
NROW = 42                      # 40 angles + x + y rows in the transposed block

PI = float(np.pi)

# CZ ring combined sign diagonal (wire i <-> amp bit 3-i, wire0 = MSB)
_bits = ((np.arange(16)[None, :] >> (3 - np.arange(4)[:, None])) & 1)
_sig = np.ones(16)
for (_i, _j) in [(0, 1), (1, 2), (2, 3), (3, 0)]:
    _sig *= np.where((_bits[_i] == 1) & (_bits[_j] == 1), -1.0, 1.0)
CZ_SIG = _sig  # 16 values of +-1
POPCNT = np.array([bin(k).count("1") for k in range(16)])


def build_bass(state_bf16=False):
    """Emit the per-core Bass program (same program for all 8 cores)."""
    SDT = BF16 if state_bf16 else F32
    nc = bacc.Bacc("TRN2", target_bir_lowering=False, debug=False,
                   enable_asserts=False)

    xy = nc.dram_tensor("xy", [N, 2], F32, kind="ExternalInput").ap()
    w1 = nc.dram_tensor("W1", [2, 16], F32, kind="ExternalInput").ap()
    b1 = nc.dram_tensor("b1", [16], F32, kind="ExternalInput").ap()
    w2 = nc.dram_tensor("W2", [16, 40], F32, kind="ExternalInput").ap()
    b2 = nc.dram_tensor("b2", [40], F32, kind="ExternalInput").ap()
    w3 = nc.dram_tensor("W3", [4, 8], F32, kind="ExternalInput").ap()
    b3 = nc.dram_tensor("b3", [8], F32, kind="ExternalInput").ap()
    w4 = nc.dram_tensor("W4", [8, 1], F32, kind="ExternalInput").ap()
    b4 = nc.dram_tensor("b4", [1], F32, kind="ExternalInput").ap()
    out_d = nc.dram_tensor("out", [N, 1], F32, kind="ExternalOutput").ap()

    with tile.TileContext(nc) as tc:
        with (
            tc.tile_pool(name="consts", bufs=1) as cpool,
            tc.tile_pool(name="persist", bufs=1) as pp,
            tc.tile_pool(name="front", bufs=2) as fp,
            tc.tile_pool(name="scratch", bufs=2) as sp,
            tc.tile_pool(name="small", bufs=1) as kp,
            tc.tile_pool(name="psum_mm", bufs=2, space="PSUM") as qmm,
            tc.tile_pool(name="psum_tr", bufs=2, space="PSUM") as qtr,
        ):
            # ---------------- constants ----------------
            ident = cpool.tile([128, 128], F32)
            masks.make_identity(nc, ident[:])
            halfpi = cpool.tile([128, 1], F32)
            nc.vector.memset(halfpi[:], PI / 2)

            w1aug = cpool.tile([3, 16], F32)
            nc.sync.dma_start(w1aug[0:1, :], b1.unsqueeze(0))
            nc.sync.dma_start(w1aug[1:3, :], w1)
            w2aug = cpool.tile([17, 40], F32)
            nc.sync.dma_start(w2aug[0:16, :], w2)
            nc.sync.dma_start(w2aug[16:17, :], b2.unsqueeze(0))

            # head weights flattened then broadcast to all 128 partitions
            hrow = cpool.tile([1, 49], F32)
            nc.sync.dma_start(hrow[0:1, 0:32], w3.rearrange("a b -> (a b)").unsqueeze(0))
            nc.sync.dma_start(hrow[0:1, 32:40], b3.unsqueeze(0))
            nc.sync.dma_start(hrow[0:1, 40:48], w4.rearrange("a b -> (a b)").unsqueeze(0))
            nc.sync.dma_start(hrow[0:1, 48:49], b4.unsqueeze(0))
            ones_r = cpool.tile([1, 128], F32)
            nc.vector.memset(ones_r[:], 1.0)
            onesrow = cpool.tile([1, CH * 128], F32)
            nc.vector.memset(onesrow[:], 1.0)
            hw_ps = qtr.tile([128, 49], F32, tag="misc", bufs=1)
            nc.tensor.matmul(hw_ps[:], ones_r[:], hrow[:])
            hwb = cpool.tile([128, 49], F32)
            nc.scalar.copy(hwb[:], hw_ps[:])
            # fold the (1/4)^2 init-state scale into W3
            nc.vector.tensor_scalar(hwb[:, 0:32], hwb[:, 0:32], 1.0 / 16.0, None,
                                    OP.mult)

            def hcol(j):  # [128,1] per-partition scalar views of the head weights
                return hwb[:, j:j + 1]

            # CZ combined sign tensor, state-shaped
            sig = cpool.tile([128, 32 * M], SDT)
            nc.vector.memset(sig[:], 1.0)
            for k in range(16):
                if CZ_SIG[k] < 0:
                    nc.vector.memset(sig[:, k * M:(k + 1) * M], -1.0)
                    nc.vector.memset(sig[:, (16 + k) * M:(17 + k) * M], -1.0)

            # ---------------- persistent batch-major tensors ----------------
            tt = pp.tile([128, NANG * M], SDT)    # tan(theta/2)
            ntt = pp.tile([128, NANG * M], SDT)   # -tan(theta/2)
            cprod = pp.tile([128, M], F32)        # prod_j cos(theta_j/2)
            xb = pp.tile([128, M], F32)           # x, batch-major
            yb = pp.tile([128, M], F32)
            state = pp.tile([128, 32 * M], SDT)
            tmp = pp.tile([128, 32 * M], SDT)

            # ---------------- front end, chunked over m-blocks ----------------
            for c in range(NCHUNK):
                b0 = c * CH * 128             # first element of chunk
                nb = CH * 128                 # elements in chunk
                xyc = fp.tile([3, nb], F32, tag="xyc")
                nc.vector.memset(xyc[0:1, :], 1.0)
                nc.sync.dma_start(xyc[1:2, :],
                                  xy[b0:b0 + nb, 0:1].rearrange("n o -> o n"))
                nc.sync.dma_start(xyc[2:3, :],
                                  xy[b0:b0 + nb, 1:2].rearrange("n o -> o n"))

                htc = fp.tile([17, nb], F32, tag="htc")
                nc.sync.dma_start(htc[16:17, :], onesrow[:])
                for q in range(nb // 512):
                    hps = qmm.tile([16, 512], F32, tag="hps")
                    nc.tensor.matmul(hps[:], w1aug[:], xyc[:, q * 512:(q + 1) * 512])
                    nc.scalar.activation(htc[0:16, q * 512:(q + 1) * 512], hps[:],
                                         AF.Tanh)

                ppc = fp.tile([42, nb], F32, tag="ppc")
                nc.sync.dma_start(ppc[40:41, :],
                                  xy[b0:b0 + nb, 0:1].rearrange("n o -> o n"))
                nc.sync.dma_start(ppc[41:42, :],
                                  xy[b0:b0 + nb, 1:2].rearrange("n o -> o n"))
                for q in range(nb // 512):
                    pps = qmm.tile([40, 512], F32, tag="pps")
                    nc.tensor.matmul(pps[:], w2aug[:], htc[:, q * 512:(q + 1) * 512])
                    nc.scalar.copy(ppc[0:40, q * 512:(q + 1) * 512], pps[:])

                # transpose the chunk's CH blocks of [42,128] -> [128,42]
                tblk = sp.tile([128, CH * NROW], F32, tag="tblk")
                half = CH // 2  # blocks per PSUM bank tile (<=512 f32)
                for g in range(2):
                    tps = qtr.tile([128, half * NROW], F32, tag="tps")
                    for u in range(half):
                        i = g * half + u
                        nc.tensor.transpose(
                            tps[:, u * NROW:(u + 1) * NROW],
                            ppc[:, i * 128:(i + 1) * 128],
                            ident[0:42, 0:42],
                        )
                    nc.scalar.copy(tblk[:, g * half * NROW:(g + 1) * half * NROW],
                                   tps[:])

                # tblk[lane, mc*42 + j]: j<40 pre-activation angles, 40=x, 41=y
                t3 = tblk.rearrange("p (mc j) -> p mc j", j=NROW)
                # x/y for this chunk (strided copy, small)
                nc.vector.tensor_copy(xb[:, c * CH:(c + 1) * CH], t3[:, :, 40])
                nc.vector.tensor_copy(yb[:, c * CH:(c + 1) * CH], t3[:, :, 41])

                # theta = tanh(pre), compact layout th[lane, j*CH + mc]
                th = sp.tile([128, NANG * CH], F32, tag="th")
                th3 = th.rearrange("p (j mc) -> p mc j", mc=CH)
                nc.scalar.activation(th3, t3[:, :, 0:40], AF.Tanh)

                # cos(theta/2) and per-chunk product over j
                cosc = sp.tile([128, NANG * CH], F32, tag="cosc")
                nc.scalar.activation(cosc[:], th[:], AF.Sin, scale=0.5, bias=halfpi[:])
                # product tree over the 40 j-blocks (each block CH cols)
                r32 = sp.tile([128, 16 * CH], F32, tag="r32")
                nc.vector.tensor_mul(r32[:], cosc[:, 0:16 * CH],
                                     cosc[:, 16 * CH:32 * CH])
                r8 = sp.tile([128, 8 * CH], F32, tag="r8")
                nc.vector.tensor_mul(r8[:], r32[:, 0:8 * CH], r32[:, 8 * CH:16 * CH])
                nc.vector.tensor_mul(r8[:], r8[:], cosc[:, 32 * CH:40 * CH])
                r4 = sp.tile([128, 4 * CH], F32, tag="r4")
                nc.vector.tensor_mul(r4[:], r8[:, 0:4 * CH], r8[:, 4 * CH:8 * CH])
                r2 = sp.tile([128, 2 * CH], F32, tag="r2")
                nc.vector.tensor_mul(r2[:], r4[:, 0:2 * CH], r4[:, 2 * CH:4 * CH])
                nc.vector.tensor_mul(cprod[:, c * CH:(c + 1) * CH],
                                     r2[:, 0:CH], r2[:, CH:2 * CH])

                # tan(theta/2) = x*(((c7*u + c5)*u + c3)*u + 1), x = theta/2
                xt = sp.tile([128, NANG * CH], F32, tag="xt")
                nc.vector.tensor_scalar(xt[:], th[:], 0.5, None, OP.mult)
                ut = sp.tile([128, NANG * CH], F32, tag="ut")
                nc.scalar.activation(ut[:], xt[:], AF.Square)
                vt = sp.tile([128, NANG * CH], F32, tag="vt")
                nc.vector.tensor_scalar(vt[:], ut[:], 17.0 / 315.0, 2.0 / 15.0,
                                        OP.mult, OP.add)
                nc.vector.tensor_mul(vt[:], vt[:], ut[:])
                nc.vector.tensor_scalar(vt[:], vt[:], 1.0 / 3.0, None, OP.add)
                nc.vector.tensor_mul(vt[:], vt[:], ut[:])
                nc.vector.tensor_scalar(vt[:], vt[:], 1.0, None, OP.add)
                nc.vector.tensor_mul(xt[:], vt[:], xt[:])

                # scatter into the full-M tan tensors (+ and -)
                tt3 = tt.rearrange("p (j m) -> p j m", m=M)[:, :, c * CH:(c + 1) * CH]
                ntt3 = ntt.rearrange("p (j m) -> p j m", m=M)[:, :, c * CH:(c + 1) * CH]
                xt3 = xt.rearrange("p (j mc) -> p j mc", mc=CH)
                nc.vector.tensor_copy(tt3, xt3)
                nc.vector.tensor_scalar(ntt3, xt3, -1.0, None, OP.mult)

            # ---------------- closed-form init state ----------------
            cx = kp.tile([128, M], F32)
            nc.scalar.activation(cx[:], xb[:], AF.Sin, scale=PI / 2, bias=halfpi[:])
            sx = kp.tile([128, M], F32)
            nc.scalar.activation(sx[:], xb[:], AF.Sin, scale=PI / 2)
            av = kp.tile([128, M], F32)
            nc.vector.tensor_sub(av[:], cx[:], sx[:])
            bv = kp.tile([128, M], F32)
            nc.vector.tensor_add(bv[:], cx[:], sx[:])
            a2 = kp.tile([128, M], F32)
            nc.scalar.activation(a2[:], av[:], AF.Square)
            bsq = kp.tile([128, M], F32)
            nc.scalar.activation(bsq[:], bv[:], AF.Square)
            abv = kp.tile([128, M], F32)
            nc.vector.tensor_mul(abv[:], av[:], bv[:])
            r_n = []
            for n, (l, r) in enumerate([(a2, a2), (a2, abv), (a2, bsq),
                                        (abv, bsq), (bsq, bsq)]):
                rn = kp.tile([128, M], F32, name=f"r_{n}")
                nc.vector.tensor_mul(rn[:], l[:], r[:])
                r_n.append(rn)

            sy = kp.tile([128, M], F32)
            nc.scalar.activation(sy[:], yb[:], AF.Sin, scale=PI / 2)
            cy = kp.tile([128, M], F32)
            nc.scalar.activation(cy[:], yb[:], AF.Sin, scale=PI / 2, bias=halfpi[:])
            u_y = kp.tile([128, M], F32)
            nc.scalar.activation(u_y[:], sy[:], AF.Square)
            cphi = kp.tile([128, M], F32)
            nc.vector.tensor_scalar(cphi[:], u_y[:], -2.0, 1.0, OP.mult, OP.add)
            sphi = kp.tile([128, M], F32)
            nc.vector.tensor_mul(sphi[:], sy[:], cy[:])
            nc.vector.tensor_scalar(sphi[:], sphi[:], 2.0, None, OP.mult)
            u_c = kp.tile([128, M], F32)
            nc.scalar.activation(u_c[:], cphi[:], AF.Square)
            c2phi = kp.tile([128, M], F32)
            nc.vector.tensor_scalar(c2phi[:], u_c[:], 2.0, -1.0, OP.mult, OP.add)
            s2phi = kp.tile([128, M], F32)
            nc.vector.tensor_mul(s2phi[:], sphi[:], cphi[:])
            nc.vector.tensor_scalar(s2phi[:], s2phi[:], 2.0, None, OP.mult)
            nsphi = kp.tile([128, M], F32)
            nc.vector.tensor_scalar(nsphi[:], sphi[:], -1.0, None, OP.mult)
            ns2phi = kp.tile([128, M], F32)
            nc.vector.tensor_scalar(ns2phi[:], s2phi[:], -1.0, None, OP.mult)

            cos_n = [c2phi, cphi, None, cphi, c2phi]
            sin_n = [ns2phi, nsphi, None, sphi, s2phi]
            for k in range(16):
                n = int(POPCNT[k])
                re_sl = state[:, k * M:(k + 1) * M]
                im_sl = state[:, (16 + k) * M:(17 + k) * M]
                if n == 2:
                    nc.vector.tensor_copy(re_sl, r_n[2][:])
                    nc.vector.memset(im_sl, 0.0)
                else:
                    nc.vector.tensor_mul(re_sl, r_n[n][:], cos_n[n][:])
                    nc.vector.tensor_mul(im_sl, r_n[n][:], sin_n[n][:])

            # ---------------- gate loop ----------------
            def gate(kind, wire, j):
                """state += (+-t_j) * P(state); P = amp-bit-(3-wire) swap,
                with re/im swap for RX."""
                p = 3 - wire              # amp bit position
                hi, lo = 1 << (3 - p), 1 << p
                st6 = state.rearrange("p (b4 h bj l m) -> p b4 h bj l m",
                                      b4=2, h=hi, bj=2, l=lo, m=M)
                tm6 = tmp.rearrange("p (b4 h bj l m) -> p b4 h bj l m",
                                    b4=2, h=hi, bj=2, l=lo, m=M)
                tt3 = tt.rearrange("p (j m) -> p j m", m=M)
                ntt3 = ntt.rearrange("p (j m) -> p j m", m=M)
                for qb4 in range(2):
                    for qbj in range(2):
                        if kind == "rx":
                            src = st6[:, 1 - qb4, :, 1 - qbj, :, :]
                            tsel = tt3 if qb4 == 0 else ntt3
                        else:  # ry: sign by target amp bit, no re/im mix
                            src = st6[:, qb4, :, 1 - qbj, :, :]
                            tsel = ntt3 if qbj == 0 else tt3
                        tv = tsel[:, j, :].unsqueeze(1).unsqueeze(1)
                        tv = tv.broadcast_to((128, hi, lo, M))
                        nc.vector.tensor_mul(tm6[:, qb4, :, qbj, :, :], tv, src)
                nc.vector.tensor_add(state[:], state[:], tmp[:])

            for l in range(5):
                for i in range(4):
                    gate("rx", i, l * 8 + i)
                    gate("ry", i, l * 8 + i + 4)
                nc.vector.tensor_mul(state[:], state[:], sig[:])

            # ---------------- readout ----------------
            sq = tmp if not state_bf16 else kp.tile([128, 32 * M], F32, name="sqf")
            nc.scalar.activation(sq[:], state[:], AF.Square)
            pr = kp.tile([128, 16 * M], F32)
            nc.vector.tensor_add(pr[:], sq[:, 0:16 * M], sq[:, 16 * M:32 * M])

            # Walsh: level trees over amp blocks
            pr3 = pr.rearrange("p (k2 two m) -> p k2 two m", two=2, m=M)
            s1 = kp.tile([128, 8 * M], F32)
            d1 = kp.tile([128, 8 * M], F32)
            s13 = s1.rearrange("p (k m) -> p k m", m=M)
            d13 = d1.rearrange("p (k m) -> p k m", m=M)
            nc.vector.tensor_add(s13, pr3[:, :, 0, :], pr3[:, :, 1, :])
            nc.vector.tensor_sub(d13, pr3[:, :, 0, :], pr3[:, :, 1, :])

            s2 = kp.tile([128, 4 * M], F32)
            d2 = kp.tile([128, 4 * M], F32)
            s1q = s1.rearrange("p (k2 two m) -> p k2 two m", two=2, m=M)
            nc.vector.tensor_add(s2.rearrange("p (k m) -> p k m", m=M),
                                 s1q[:, :, 0, :], s1q[:, :, 1, :])
            nc.vector.tensor_sub(d2.rearrange("p (k m) -> p k m", m=M),
                                 s1q[:, :, 0, :], s1q[:, :, 1, :])

            s3 = kp.tile([128, 2 * M], F32)
            d3 = kp.tile([128, 2 * M], F32)
            s2q = s2.rearrange("p (k2 two m) -> p k2 two m", two=2, m=M)
            nc.vector.tensor_add(s3.rearrange("p (k m) -> p k m", m=M),
                                 s2q[:, :, 0, :], s2q[:, :, 1, :])
            nc.vector.tensor_sub(d3.rearrange("p (k m) -> p k m", m=M),
                                 s2q[:, :, 0, :], s2q[:, :, 1, :])

            qs = [kp.tile([128, M], F32, name=f"q_{i}") for i in range(4)]
            # wire0 (MSB bit b3): s3[0] - s3[1]
            nc.vector.tensor_sub(qs[0][:], s3[:, 0:M], s3[:, M:2 * M])
            # wire1 (b2): d3[0] + d3[1]
            nc.vector.tensor_add(qs[1][:], d3[:, 0:M], d3[:, M:2 * M])
            # wire2 (b1): sum of 4 d2 blocks
            t2a = kp.tile([128, 2 * M], F32)
            nc.vector.tensor_add(t2a[:], d2[:, 0:2 * M], d2[:, 2 * M:4 * M])
            nc.vector.tensor_add(qs[2][:], t2a[:, 0:M], t2a[:, M:2 * M])
            # wire3 (b0): sum of 8 d1 blocks
            t1a = kp.tile([128, 4 * M], F32)
            nc.vector.tensor_add(t1a[:], d1[:, 0:4 * M], d1[:, 4 * M:8 * M])
            t1b = kp.tile([128, 2 * M], F32)
            nc.vector.tensor_add(t1b[:], t1a[:, 0:2 * M], t1a[:, 2 * M:4 * M])
            nc.vector.tensor_add(qs[3][:], t1b[:, 0:M], t1b[:, M:2 * M])

            # scale by C^2 (the 1/16 is folded into W3)
            c2t = kp.tile([128, M], F32)
            nc.scalar.activation(c2t[:], cprod[:], AF.Square)
            for i in range(4):
                nc.vector.tensor_mul(qs[i][:], qs[i][:], c2t[:])

            # ---------------- head MLP ----------------
            outt = kp.tile([128, M], F32)
            macc = kp.tile([128, M], F32)
            zt = kp.tile([128, M], F32)
            for jf in range(8):
                nc.vector.tensor_scalar(zt[:], qs[0][:], hcol(0 * 8 + jf),
                                        hcol(32 + jf), OP.mult, OP.add)
                for i in range(1, 4):
                    nc.vector.tensor_scalar(macc[:], qs[i][:], hcol(i * 8 + jf),
                                            None, OP.mult)
                    nc.vector.tensor_add(zt[:], zt[:], macc[:])
                nc.scalar.activation(zt[:], zt[:], AF.Tanh)
                if jf == 0:
                    nc.vector.tensor_scalar(outt[:], zt[:], hcol(40 + 0),
                                            hcol(48), OP.mult, OP.add)
                else:
                    nc.vector.tensor_scalar(macc[:], zt[:], hcol(40 + jf),
                                            None, OP.mult)
                    nc.vector.tensor_add(outt[:], outt[:], macc[:])

            # ---------------- output transpose + store ----------------
            ops_ = qtr.tile([128, 128], F32, tag="misc", bufs=1)
            nc.tensor.transpose(ops_[:], outt[:], ident[:])
            otr = kp.tile([128, 128], F32)
            nc.scalar.copy(otr[:], ops_[:])
            nc.sync.dma_start(out_d.rearrange("(m l) o -> m (l o)", m=M), otr[:])

    nc.compile()
    return nc


_CACHE = {}


def _get_nc(state_bf16=False):
    key = state_bf16
    if key not in _CACHE:
        _CACHE[key] = build_bass(state_bf16)
    return _CACHE[key]


def kernel(xy, W1, b1, W2, b2, W3, b3, W4, b4):
    nc = _get_nc()
    xy = np.ascontiguousarray(np.asarray(xy, dtype=np.float32))
    weights = dict(W1=W1, b1=b1, W2=W2, b2=b2, W3=W3, b3=b3, W4=W4, b4=b4)
    weights = {k: np.ascontiguousarray(np.asarray(v, dtype=np.float32))
               for k, v in weights.items()}
    in_maps = [
        {"xy": xy[c * N:(c + 1) * N], **weights}
        for c in range(N_CORES)
    ]
    res = bass_utils.run_bass_kernel_spmd(nc, in_maps, list(range(N_CORES)))
    return np.concatenate([res.results[c]["out"] for c in range(N_CORES)], axis=0)
